# revision 7
# baseline (speedup 1.0000x reference)
"""Bahdanau pointer-attention kernel for Trainium2 (8 NeuronCores, SPMD).

Computes energy[b, 1, n] = V . tanh(x[b, :N] @ W1.T + x[b, -1] @ W2.T)
for B=32, N=2048, D=1024.

Sharding: data-parallel over batch B across 8 cores (4 batches/core).
Per-core layout: contraction over d requires d on SBUF partitions, so the
host pre-transposes each core's x shard to [D, 4*N] during sharding.

Per-core pipeline (Tile framework):
  - keys matmul: psum[e128, n512] += W1T[d128, e128].T @ xT[d128, n512]
    (both operands float32r - 1 PE pass at full rate, ~fp22 precision)
  - ACT: tanh(psum + query_bias) fused via activation bias (per-partition)
  - V-dot: psum[1, n512] += VT[e128, 1].T @ tanh[e128, n512] on PE (bf16)
  - query preamble: psum[e128, b4] += W2T[d128, e128].T @ xqT[d128, b4]
"""

from contextlib import ExitStack

import numpy as np
import ml_dtypes

import concourse.bass as bass
import concourse.mybir as mybir
import concourse.tile as tile
from concourse import bacc
from concourse.bass_utils import run_bass_kernel_spmd

B, N, D = 32, 2048, 1024
CORES = 8
BPC = B // CORES            # batches per core
NTOT = BPC * N              # 8192 key positions per core
P = 128
DC = D // P                 # 8 d-chunks (contraction)
EC = D // P                 # 8 e-chunks (output feature)
NT = 512                    # n tile (one PSUM bank of f32)
NCH = NTOT // NT            # 16 n-chunks per core
NPB = N // NT               # n-chunks per batch

f32 = mybir.dt.float32
f32r = mybir.dt.float32r
bf16 = mybir.dt.bfloat16

TRACE = False
LAST_EXEC_NS = None
LAST_RESULTS = None

_NC_CACHE = None


def _w_slice(w_sb, dc, ec):
    return w_sb[:, dc, ec * P:(ec + 1) * P]


def _body(ctx, tc, xT, xqT, w1T, w2T, vT, out):
    nc = tc.nc
    Tanh = mybir.ActivationFunctionType.Tanh

    w_pool = ctx.enter_context(tc.tile_pool(name="w", bufs=1))
    x_pool = ctx.enter_context(tc.tile_pool(name="x", bufs=3))
    t_pool = ctx.enter_context(tc.tile_pool(name="tanh", bufs=3 * EC))
    small = ctx.enter_context(tc.tile_pool(name="small", bufs=1))
    en_pool = ctx.enter_context(tc.tile_pool(name="en", bufs=3))
    kpsum = ctx.enter_context(tc.tile_pool(name="kpsum", bufs=3, space="PSUM"))
    vpsum = ctx.enter_context(tc.tile_pool(name="vpsum", bufs=2, space="PSUM"))
    qpsum = ctx.enter_context(tc.tile_pool(name="qpsum", bufs=2, space="PSUM"))

    # Resident weights, d-chunk on partitions: [p=128, (c, e)]
    w1_sb = w_pool.tile([P, DC, D], f32r, tag="w1")
    nc.sync.dma_start(w1_sb[:], w1T.rearrange("(c p) e -> p c e", p=P))
    w2_sb = w_pool.tile([P, DC, D], f32r, tag="w2")
    nc.sync.dma_start(w2_sb[:], w2T.rearrange("(c p) e -> p c e", p=P))
    v_sb = small.tile([P, EC], bf16, tag="v")
    nc.sync.dma_start(v_sb[:], vT[:, :])
    xq_sb = small.tile([P, DC, BPC], f32r, tag="xq")
    nc.sync.dma_start(xq_sb[:], xqT.rearrange("(c p) b -> p c b", p=P))

    # Query preamble: q_sb[e128, (ec, b)] = x_query @ W2.T  (transposed)
    q_sb = small.tile([P, EC * BPC], f32, tag="q")
    for ec in range(EC):
        pq = qpsum.tile([P, BPC], f32)
        for dc in range(DC):
            nc.tensor.matmul(
                pq[:],
                lhsT=_w_slice(w2_sb, dc, ec),
                rhs=xq_sb[:, dc, :],
                start=(dc == 0),
                stop=(dc == DC - 1),
            )
        nc.vector.tensor_copy(q_sb[:, ec * BPC:(ec + 1) * BPC], pq[:])

    # Main loop, software-pipelined: V-dot for chunk k-1 is emitted after
    # the keys matmuls of chunk k so the PE never waits on ACT.
    pending = None  # (tanh tiles, chunk index)
    for ch in range(NCH):
        b = ch // NPB
        x_sb = x_pool.tile([P, DC, NT], f32r, tag="x")
        nc.sync.dma_start(
            x_sb[:],
            xT.rearrange("(c p) n -> p c n", p=P)[:, :, ch * NT:(ch + 1) * NT],
        )
        tts = []
        for ec in range(EC):
            pk = kpsum.tile([P, NT], f32)
            for dc in range(DC):
                nc.tensor.matmul(
                    pk[:],
                    lhsT=_w_slice(w1_sb, dc, ec),
                    rhs=x_sb[:, dc, :],
                    start=(dc == 0),
                    stop=(dc == DC - 1),
                )
            tt = t_pool.tile([P, NT], bf16, tag="tanh")
            nc.scalar.activation(
                tt[:], pk[:], Tanh,
                bias=q_sb[:, ec * BPC + b: ec * BPC + b + 1],
            )
            tts.append(tt)
        if pending is not None:
            _emit_vdot(nc, vpsum, en_pool, v_sb, out, *pending)
        pending = (tts, ch)
    _emit_vdot(nc, vpsum, en_pool, v_sb, out, *pending)


def _emit_vdot(nc, vpsum, en_pool, v_sb, out, tts, ch):
    pv = vpsum.tile([1, NT], f32)
    for ec in range(EC):
        nc.tensor.matmul(
            pv[:],
            lhsT=v_sb[:, ec:ec + 1],
            rhs=tts[ec][:],
            start=(ec == 0),
            stop=(ec == EC - 1),
        )
    en = en_pool.tile([1, NT], f32, tag="en")
    nc.vector.tensor_copy(en[:], pv[:])
    nc.sync.dma_start(out[:, ch * NT:(ch + 1) * NT], en[:])


def build_module():
    global _NC_CACHE
    if _NC_CACHE is not None:
        return _NC_CACHE
    nc = bacc.Bacc("TRN2", target_bir_lowering=False, debug=False)
    xT = nc.declare_dram_parameter("xT", [D, NTOT], f32r, isOutput=False)
    xqT = nc.declare_dram_parameter("xqT", [D, BPC], f32r, isOutput=False)
    w1T = nc.declare_dram_parameter("w1T", [D, D], f32r, isOutput=False)
    w2T = nc.declare_dram_parameter("w2T", [D, D], f32r, isOutput=False)
    vT = nc.declare_dram_parameter("vT", [P, EC], bf16, isOutput=False)
    out = nc.declare_dram_parameter("out", [1, NTOT], f32, isOutput=True)
    with tile.TileContext(nc) as tc:
        with ExitStack() as ctx:
            _body(ctx, tc, xT, xqT, w1T, w2T, vT, out)
    nc.compile()
    _NC_CACHE = nc
    return nc


def shard_inputs(x, W1, W2, V):
    """Host-side sharding + layout transforms. Returns per-core input maps."""
    x = np.asarray(x, dtype=np.float32)
    bf = ml_dtypes.bfloat16
    w1T = np.ascontiguousarray(np.asarray(W1, np.float32).T)
    w2T = np.ascontiguousarray(np.asarray(W2, np.float32).T)
    vT = np.ascontiguousarray(np.asarray(V, np.float32).reshape(EC, P).T).astype(bf)
    in_maps = []
    for c in range(CORES):
        xs = x[c * BPC:(c + 1) * BPC, :N, :]          # [BPC, N, D]
        xT = np.ascontiguousarray(xs.transpose(2, 0, 1)).reshape(D, NTOT)
        xq = x[c * BPC:(c + 1) * BPC, N, :]           # [BPC, D]
        xqT = np.ascontiguousarray(xq.T)              # [D, BPC]
        in_maps.append({
            "xT": xT, "xqT": xqT,
            "w1T": w1T, "w2T": w2T, "vT": vT,
        })
    return in_maps


def kernel(x, W1, W2, V, city_count):
    global LAST_EXEC_NS, LAST_RESULTS
    assert int(city_count) == N
    nc = build_module()
    in_maps = shard_inputs(x, W1, W2, V)
    res = run_bass_kernel_spmd(nc, in_maps, core_ids=list(range(CORES)),
                               trace=TRACE)
    LAST_EXEC_NS = res.exec_time_ns
    LAST_RESULTS = res
    out = np.concatenate(
        [res.results[c]["out"].reshape(BPC, N) for c in range(CORES)], axis=0
    )
    return out[:, None, :].astype(np.float32)


# revision 11
# speedup vs baseline: 3.6174x; 3.6174x over previous
"""Bahdanau pointer-attention kernel for Trainium2 (8 NeuronCores, SPMD).

Computes energy[b, 1, n] = V . tanh(x[b, :N] @ W1.T + x[b, -1] @ W2.T)
for B=32, N=2048, D=1024.

Sharding: data-parallel over batch B across 8 cores (4 batches/core).
Per-core layout: contraction over d requires d on SBUF partitions, so the
host pre-transposes each core's x shard to [D, 4*N] during sharding.

Per-core pipeline (Tile framework):
  - keys matmul: psum[e128, n512] += W1T[d128, e128].T @ xT[d128, n512]
    (both operands float32r - 1 PE pass at full rate, ~fp22 precision)
  - ACT: tanh(psum + query_bias) fused via activation bias (per-partition)
  - V-dot: psum[1, n512] += VT[e128, 1].T @ tanh[e128, n512] on PE (bf16)
  - query preamble: psum[e128, b4] += W2T[d128, e128].T @ xqT[d128, b4]
"""

from contextlib import ExitStack

import numpy as np
import ml_dtypes

import concourse.bass as bass
import concourse.mybir as mybir
import concourse.tile as tile
from concourse import bacc
from concourse.bass_utils import run_bass_kernel_spmd

B, N, D = 32, 2048, 1024
CORES = 8
BPC = B // CORES            # batches per core
NTOT = BPC * N              # 8192 key positions per core
P = 128
DC = D // P                 # 8 d-chunks (contraction)
EC = D // P                 # 8 e-chunks (output feature)
NT = 512                    # n tile (one PSUM bank of f32)
NCH = NTOT // NT            # 16 n-chunks per core
NPB = N // NT               # n-chunks per batch

f32 = mybir.dt.float32
f32r = mybir.dt.float32r
bf16 = mybir.dt.bfloat16

TRACE = False
LAST_EXEC_NS = None
LAST_RESULTS = None

_NC_CACHE = {}


def _w_slice(w_sb, dc, ec):
    return w_sb[:, dc, ec * P:(ec + 1) * P]


def _body(ctx, tc, xT, xqT, w1T, w2T, vT, out, reps=1):
    nc = tc.nc
    Tanh = mybir.ActivationFunctionType.Tanh

    w_pool = ctx.enter_context(tc.tile_pool(name="w", bufs=1))
    x_pool = ctx.enter_context(tc.tile_pool(name="x", bufs=3))
    t_pool = ctx.enter_context(tc.tile_pool(name="tanh", bufs=3 * EC))
    small = ctx.enter_context(tc.tile_pool(name="small", bufs=1))
    en_pool = ctx.enter_context(tc.tile_pool(name="en", bufs=3))
    kpsum = ctx.enter_context(tc.tile_pool(name="kpsum", bufs=3, space="PSUM"))
    vpsum = ctx.enter_context(tc.tile_pool(name="vpsum", bufs=2, space="PSUM"))
    qpsum = ctx.enter_context(tc.tile_pool(name="qpsum", bufs=2, space="PSUM"))

    # Resident weights, d-chunk on partitions: [p=128, (c, e)]
    w1_sb = w_pool.tile([P, DC, D], f32r, tag="w1")
    nc.sync.dma_start(w1_sb[:], w1T.rearrange("(c p) e -> p c e", p=P))
    w2_sb = w_pool.tile([P, DC, D], f32r, tag="w2")
    nc.sync.dma_start(w2_sb[:], w2T.rearrange("(c p) e -> p c e", p=P))
    v_sb = small.tile([P, EC], bf16, tag="v")
    nc.sync.dma_start(v_sb[:], vT[:, :])
    xq_sb = small.tile([P, DC, BPC], f32r, tag="xq")
    nc.sync.dma_start(xq_sb[:], xqT.rearrange("(c p) b -> p c b", p=P))

    # Query preamble: q_sb[e128, (ec, b)] = x_query @ W2.T  (transposed)
    q_sb = small.tile([P, EC * BPC], f32, tag="q")
    for ec in range(EC):
        pq = qpsum.tile([P, BPC], f32)
        for dc in range(DC):
            nc.tensor.matmul(
                pq[:],
                lhsT=_w_slice(w2_sb, dc, ec),
                rhs=xq_sb[:, dc, :],
                start=(dc == 0),
                stop=(dc == DC - 1),
            )
        nc.vector.tensor_copy(q_sb[:, ec * BPC:(ec + 1) * BPC], pq[:])

    # Main loop, software-pipelined: V-dot for chunk k-1 is emitted after
    # the keys matmuls of chunk k so the PE never waits on ACT.
    pending = None  # (tanh tiles, chunk index)
    for rep_ch in range(reps * NCH):
        ch = rep_ch % NCH
        b = ch // NPB
        x_sb = x_pool.tile([P, DC, NT], f32r, tag="x")
        nc.sync.dma_start(
            x_sb[:],
            xT.rearrange("(c p) n -> p c n", p=P)[:, :, ch * NT:(ch + 1) * NT],
        )
        tts = []
        for ec in range(EC):
            pk = kpsum.tile([P, NT], f32)
            for dc in range(DC):
                nc.tensor.matmul(
                    pk[:],
                    lhsT=_w_slice(w1_sb, dc, ec),
                    rhs=x_sb[:, dc, :],
                    start=(dc == 0),
                    stop=(dc == DC - 1),
                )
            tt = t_pool.tile([P, NT], bf16, tag="tanh")
            nc.scalar.activation(
                tt[:], pk[:], Tanh,
                bias=q_sb[:, ec * BPC + b: ec * BPC + b + 1],
            )
            tts.append(tt)
        if pending is not None:
            _emit_vdot(nc, vpsum, en_pool, v_sb, out, *pending)
        pending = (tts, ch)
    _emit_vdot(nc, vpsum, en_pool, v_sb, out, *pending)


def _emit_vdot(nc, vpsum, en_pool, v_sb, out, tts, ch):
    pv = vpsum.tile([1, NT], f32)
    for ec in range(EC):
        nc.tensor.matmul(
            pv[:],
            lhsT=v_sb[:, ec:ec + 1],
            rhs=tts[ec][:],
            start=(ec == 0),
            stop=(ec == EC - 1),
        )
    en = en_pool.tile([1, NT], f32, tag="en")
    nc.vector.tensor_copy(en[:], pv[:])
    nc.sync.dma_start(out[:, ch * NT:(ch + 1) * NT], en[:])


def build_module(reps=1):
    if reps in _NC_CACHE:
        return _NC_CACHE[reps]
    nc = bacc.Bacc("TRN2", target_bir_lowering=False, debug=False)
    xT = nc.declare_dram_parameter("xT", [D, NTOT], f32r, isOutput=False)
    xqT = nc.declare_dram_parameter("xqT", [D, BPC], f32r, isOutput=False)
    w1T = nc.declare_dram_parameter("w1T", [D, D], f32r, isOutput=False)
    w2T = nc.declare_dram_parameter("w2T", [D, D], f32r, isOutput=False)
    vT = nc.declare_dram_parameter("vT", [P, EC], bf16, isOutput=False)
    out = nc.declare_dram_parameter("out", [1, NTOT], f32, isOutput=True)
    with tile.TileContext(nc) as tc:
        with ExitStack() as ctx:
            _body(ctx, tc, xT, xqT, w1T, w2T, vT, out, reps=reps)
    nc.compile()
    _NC_CACHE[reps] = nc
    return nc


def shard_inputs(x, W1, W2, V):
    """Host-side sharding + layout transforms. Returns per-core input maps."""
    x = np.asarray(x, dtype=np.float32)
    bf = ml_dtypes.bfloat16
    w1T = np.ascontiguousarray(np.asarray(W1, np.float32).T)
    w2T = np.ascontiguousarray(np.asarray(W2, np.float32).T)
    vT = np.ascontiguousarray(np.asarray(V, np.float32).reshape(EC, P).T).astype(bf)
    in_maps = []
    for c in range(CORES):
        xs = x[c * BPC:(c + 1) * BPC, :N, :]          # [BPC, N, D]
        xT = np.ascontiguousarray(xs.transpose(2, 0, 1)).reshape(D, NTOT)
        xq = x[c * BPC:(c + 1) * BPC, N, :]           # [BPC, D]
        xqT = np.ascontiguousarray(xq.T)              # [D, BPC]
        in_maps.append({
            "xT": xT, "xqT": xqT,
            "w1T": w1T, "w2T": w2T, "vT": vT,
        })
    return in_maps


def kernel(x, W1, W2, V, city_count):
    global LAST_EXEC_NS, LAST_RESULTS
    assert int(city_count) == N
    nc = build_module()
    in_maps = shard_inputs(x, W1, W2, V)
    res = run_bass_kernel_spmd(nc, in_maps, core_ids=list(range(CORES)),
                               trace=TRACE)
    LAST_EXEC_NS = res.exec_time_ns
    LAST_RESULTS = res
    out = np.concatenate(
        [res.results[c]["out"].reshape(BPC, N) for c in range(CORES)], axis=0
    )
    return out[:, None, :].astype(np.float32)
